# revision 1
# baseline (speedup 1.0000x reference)
"""Trainium2 Bass kernel for nn_AdaptivePoolingClassifier (8 NeuronCores).

Math: the reference MLP is linear up to its single ReLU, so W1..W3 fold
into one 128x128 matrix on the host:
    h   = relu(x @ Wc^T + bc)       Wc = W3 W2 W1 ; bc = W3(W2 b1+b2)+b3
    p   = h @ W4^T + b4
    out = sum_n p * softmax(alpha*p, axis=1)

Device computes pt = h @ (diag(alpha) W4)^T = alpha*(p - b4) for every
row (rows sharded 8 ways) and streams pt back to DRAM; the host finishes
the softmax pooling in f64 (num/den sums over rows) exactly as it
already finishes the fold / bias algebra.  The softmax weights are
invariant to the per-column constant alpha*b4 shift.

Device layout/schedule (v4):
  - x host-transposed to [128(feat), rows], quantized to fp8e4 (halves
    HBM traffic; weights stay bf16 - mixed-dtype matmul; row-quantization
    noise averages out over the 200k-row pooling, rel err ~1.4e-3).
  - L1: wct stationary, TILE=1280 rows split across two psum pools
    (ps_a 768 cols -> ACT relu, ps_d 512 cols -> DVE relu) so the two
    relu halves recycle their psum banks independently.
  - L4: h chunks (128 rows) as matmul stationary so pt lands
    rows-on-partitions; runs 2 tiles behind L1 so stationary loads are
    long-ready (they then pipeline at ~60 cycles/chunk).  pt accumulates
    in psum batches of 96 chunks and is DMAed straight to DRAM (f32).
"""

import numpy as np
import ml_dtypes

from concourse import bacc, mybir, tile
from concourse.bass_utils import run_bass_kernel_spmd

N_CORES = 8
N_ROWS = 200000
F = 128
OUT = 5

ROWS_PAD = 200704            # 8 * 25088
RPC = ROWS_PAD // N_CORES    # rows per core = 25088
T0 = 768                     # prologue tile (one ps_a tile)
TILE = 1280                  # steady tile: 768 (ps_a/ACT) + 512 (ps_d/DVE)
N_TILES = (RPC - T0) // TILE  # 19
A_COLS = 768                 # ACT relu cols per tile (6 chunks)
D_COLS = 512                 # DVE relu cols per tile (4 chunks)
GROUP = 3840                 # rows per steady-state DMA (3 tiles)
CHUNK = 128
N_CHUNKS = RPC // CHUNK      # 196
SLOTS = 48                   # max pt chunks per psum batch
# batch ends staggered so the last copies+DMAs overlap the PE stream
BATCH_ENDS = (48, 96, 144, 176, 196)

F32 = mybir.dt.float32
BF16 = mybir.dt.bfloat16
FP8 = mybir.dt.float8e4
AF = mybir.ActivationFunctionType
ALU = mybir.AluOpType


def build_bass(has_bias=False):
    nc = bacc.Bacc()

    CONST_COLS = (F + OUT + 1) if has_bias else (F + OUT)
    cst_ext = nc.declare_dram_parameter(
        "cst", [F, CONST_COLS], BF16, isOutput=False
    )
    xt_ext = nc.declare_dram_parameter("xt", [F, RPC], FP8, isOutput=False)
    pt_ext = nc.declare_dram_parameter(
        "pt", [F, N_CHUNKS, OUT], F32, isOutput=True
    )

    with tile.TileContext(nc) as tc:
        with (
            tc.tile_pool(name="scratch", bufs=1) as scratch,
            tc.tile_pool(name="xin", bufs=4) as xin,
            tc.tile_pool(name="hbufl", bufs=4) as hbufl,
            tc.tile_pool(name="hbufr", bufs=4) as hbufr,
            tc.tile_pool(name="ptb", bufs=2) as ptb,
            tc.tile_pool(name="ps_a", bufs=2, space="PSUM") as ps_a,
            tc.tile_pool(name="ps_d", bufs=2, space="PSUM") as ps_d,
            tc.tile_pool(name="ps_p", bufs=2, space="PSUM") as ps_p,
        ):
            cstt = scratch.tile([F, CONST_COLS], BF16)
            nc.sync.dma_start(out=cstt[:], in_=cst_ext[:])
            wct = cstt[:, :F]
            w4at = cstt[:, F : F + OUT]
            nc.tensor.ldweights(wct)  # PE observes the const DMA early
            bc = None
            if has_bias:
                bc = scratch.tile([F, 1], F32)
                nc.vector.tensor_copy(bc[:], cstt[:, F + OUT : F + OUT + 1])
            xfirst = scratch.tile([F, T0], FP8)
            nc.sync.dma_start(out=xfirst[:], in_=xt_ext[:, :T0])
            x0 = xfirst[:]

            # group 0 is exactly compute tile 0 so L1 starts ASAP
            sizes = [TILE] + [GROUP] * ((RPC - T0 - TILE) // GROUP)
            assert sum(sizes) == RPC - T0
            xg = []          # (tile, start_col_within_steady_region)
            c0 = T0
            for cw in sizes:
                t = xin.tile([F, GROUP], FP8)
                nc.sync.dma_start(out=t[:, :cw], in_=xt_ext[:, c0 : c0 + cw])
                xg.append((t, c0 - T0))
                c0 += cw

            state = {"chunk": 0, "pp": None, "bstart": 0, "bi": 0}
            hbufs = []  # per tile: (hl, hr, a_ch, n_ch)

            def act_relu(dst, src):
                if has_bias:
                    nc.scalar.activation(dst, src, AF.Relu, bias=bc[:], scale=1.0)
                else:
                    nc.scalar.activation(dst, src, AF.Relu)

            def dve_relu(dst, src):
                if has_bias:
                    nc.vector.tensor_scalar(dst, src, bc[:], 0.0, ALU.add, ALU.max)
                else:
                    nc.vector.tensor_scalar_max(dst, src, 0.0)

            def do_l1_t0():
                ha = ps_a.tile([F, A_COLS], F32, tag="ha", name="ha")
                # warmup: keep PE busy through the ACT/DVE boot window and
                # advance the clock ramp; results overwritten (start=True)
                for _ in range(3):
                    nc.tensor.matmul(ha[:, :512], wct, x0[:, :512], start=True, stop=True)
                    nc.tensor.matmul(ha[:, 512:768], wct, x0[:, 512:768], start=True, stop=True)
                nc.tensor.matmul(ha[:, :512], wct, x0[:, :512], start=True, stop=True)
                nc.tensor.matmul(ha[:, 512:768], wct, x0[:, 512:768], start=True, stop=True)
                hl = hbufl.tile([F, A_COLS], BF16, tag="hl")
                hr = hbufr.tile([F, D_COLS], BF16, tag="hr")
                act_relu(hl[:, :384], ha[:, :384])
                dve_relu(hr[:, :384], ha[:, 384:768])
                hbufs.append((hl, hr, 3, 6))

            def do_l1(rhs, shift=False):
                ha = ps_a.tile([F, A_COLS], F32, tag="ha", name="ha")
                hd = ps_d.tile([F, D_COLS], F32, tag="hd", name="hd")
                nc.tensor.matmul(ha[:, :512], wct, rhs[:, :512], start=True, stop=True)
                nc.tensor.matmul(ha[:, 512:768], wct, rhs[:, 512:768], start=True, stop=True)
                nc.tensor.matmul(hd[:], wct, rhs[:, 768:1280], start=True, stop=True)
                hl = hbufl.tile([F, A_COLS], BF16, tag="hl")
                hr = hbufr.tile([F, D_COLS], BF16, tag="hr")
                act_relu(hl[:], ha[:])
                dve_relu(hr[:], hd[:])
                hbufs.append((hl, hr, 6, 10))

            def do_l4(ti):
                hl, hr, a_ch, n_ch = hbufs[ti]
                for j in range(n_ch):
                    c = state["chunk"]
                    s = c - state["bstart"]
                    if s == 0:
                        state["pp"] = ps_p.tile(
                            [F, SLOTS, OUT], F32, tag="pp", name="pp"
                        )
                    if j < a_ch:
                        lhs = hl[:, j * CHUNK : (j + 1) * CHUNK]
                    else:
                        lhs = hr[:, (j - a_ch) * CHUNK : (j - a_ch + 1) * CHUNK]
                    nc.tensor.matmul(
                        state["pp"][:, s, :], lhs, w4at,
                        start=True, stop=True,
                    )
                    state["chunk"] = c + 1
                    if state["chunk"] in BATCH_ENDS:
                        c0 = state["bstart"]
                        n = state["chunk"] - c0
                        bi = state["bi"]
                        pts = ptb.tile([F, SLOTS, OUT], F32, tag="pts")
                        if bi % 2 == 0:
                            nc.vector.tensor_copy(
                                pts[:, :n, :], state["pp"][:, :n, :]
                            )
                        else:
                            nc.scalar.activation(
                                pts[:, :n, :], state["pp"][:, :n, :], AF.Copy,
                            )
                        nc.sync.dma_start(
                            out=pt_ext[:, c0 : state["chunk"], :],
                            in_=pts[:, :n, :],
                        )
                        state["bstart"] = state["chunk"]
                        state["bi"] = bi + 1

            do_l1_t0()
            for t in range(N_TILES):
                if t == 0:
                    gt, off = xg[0][0], 0
                else:
                    gt, off = xg[1 + (t - 1) // 3][0], ((t - 1) % 3) * TILE
                do_l1(gt[:, off : off + TILE])
                if t >= 1:
                    do_l4(t - 1)
            do_l4(N_TILES - 1)
            do_l4(N_TILES)

    nc.finalize()
    return nc


_CACHED = {}
TRACE = False
LAST = {}


def kernel(x, W1, b1, W2, b2, W3, b3, W4, b4, alpha):
    f64 = np.float64
    x2 = np.asarray(x, np.float32).reshape(N_ROWS, F)
    W1, b1, W2, b2, W3, b3, W4, b4, alpha = [
        np.asarray(a, f64) for a in (W1, b1, W2, b2, W3, b3, W4, b4, alpha)
    ]

    # fold the linear layers (exact in f64)
    Wc = W3 @ W2 @ W1
    bc = W3 @ (W2 @ b1 + b2) + b3
    alpha_safe = np.where(np.abs(alpha) < 1e-12, 1e-12, alpha)
    W4a = alpha_safe[:, None] * W4

    # pad rows to 8*25088 with zeros; pad rows dropped after the gather
    n_pad = ROWS_PAD - N_ROWS
    xp = np.concatenate([x2, np.zeros((n_pad, F), np.float32)], axis=0)
    xT = np.ascontiguousarray(xp.T).astype(ml_dtypes.float8_e4m3fn)

    has_bias = bool(np.any(bc != 0.0))
    key = ("nc", has_bias)
    if key not in _CACHED:
        _CACHED[key] = build_bass(has_bias)
    nc = _CACHED[key]

    wct_np = np.ascontiguousarray(Wc.T).astype(ml_dtypes.bfloat16)
    w4at_np = np.ascontiguousarray(W4a.T).astype(ml_dtypes.bfloat16)
    parts_list = [wct_np, w4at_np]
    if has_bias:
        parts_list.append(
            bc.reshape(F, 1).astype(np.float32).astype(ml_dtypes.bfloat16)
        )
    consts_np = np.ascontiguousarray(np.concatenate(parts_list, axis=1))

    in_maps = []
    for c in range(N_CORES):
        shard = np.ascontiguousarray(xT[:, c * RPC : (c + 1) * RPC])
        in_maps.append({"cst": consts_np, "xt": shard})

    res = run_bass_kernel_spmd(
        nc, in_maps, core_ids=list(range(N_CORES)), trace=TRACE
    )
    LAST["res"] = res

    # gather pt: per core [F(part=row-in-chunk), N_CHUNKS, OUT]
    pts = np.stack([np.asarray(r["pt"], np.float32) for r in res.results])
    # rows order: (core, chunk, partition)
    pt = pts.transpose(0, 2, 1, 3).reshape(ROWS_PAD, OUT).astype(f64)
    pt = pt[:N_ROWS]

    # host softmax pooling in f64:  out_o = sum pt*e^pt / (alpha*sum e^pt) + b4
    m = pt.max(axis=0)
    e = np.exp(pt - m)
    den = e.sum(axis=0)
    num = (pt * e).sum(axis=0)
    out = num / (alpha_safe * den) + b4
    return out[None, :].astype(np.float32)

